# revision 20
# baseline (speedup 1.0000x reference)
"""Batched sparse-dense matmul (COO SpMM) on 8 Trainium2 NeuronCores.

Problem: y[b, r] = sum_k vals[k] * x[b, cols[k]] where rows[k] == r.
  x: [128, 16384] f32, vals/rows/cols: [524288], y: [128, 8192] f32.

Strategy: at 0.39% density with a full 128-wide batch, a dense matmul
y = x @ M^T beats any per-nonzero gather on this hardware (SWDGE
descriptor generation costs ~4-9ns per gathered element — a ~300us
serial floor on the Q7 cores — while the dense stream uses the HWDGE
DMA path with no per-element work at all).  So:
  - Host: densify M^T into W [C, R] (a format conversion of the matrix,
    analogous to CSR/ELL packing), shard W's output columns across the
    8 cores (1024 rows each), and pre-tile both x^T and W for the SBUF
    partition layout.  W and x are cast to fp16 (11-bit mantissa): the
    result error is ~3e-4 relative, and the stream halves vs f32.
  - Device (per core): keep x^T resident in SBUF as 128 [128c x 128b]
    chunks (the matmul's stationary operand); stream W from HBM in 1MB
    tiles (4 c-chunks each); PSUM accumulates over the 128 c-chunks
    into y[128b x 1024r] (fp32 accumulation); copy out via DVE.
  - Host: concatenate the per-core row slices.

Set DTYPE = "f32" for an exact (2e-5 absmax) variant at ~2x the time.
"""

import sys

sys.path.insert(0, "/opt/trn_rl_repo")

import numpy as np

import concourse.bacc as bacc
import concourse.mybir as mybir
import concourse.tile as tile
from concourse.bass_utils import run_bass_kernel_spmd

B = 128        # batch
R = 8192       # rows of sparse matrix / output features
C = 16384      # cols of sparse matrix / input features
NCORES = 8
RC = R // NCORES       # rows (output features) per core
NCH = C // 128         # contraction chunks of 128
NT = RC // 512         # 512-wide PSUM column tiles per core

DTYPE = "f16"          # "f16" (fast, ~3e-4 rel err) or "f32" (exact)
_NP_DT = {"f16": np.float16, "f32": np.float32}
_MY_DT = {"f16": mybir.dt.float16, "f32": mybir.dt.float32}


def _densify_tiled(vals, rows, cols):
    """w_t[p, ch, r] = sum of vals at (row=r, col=ch*128+p): dense M^T
    pre-tiled for the SBUF partition layout, [128, NCH, R] f32."""
    w_t = np.zeros((128, NCH, R), dtype=np.float32)
    np.add.at(w_t, (cols % 128, cols // 128, rows), vals)
    return w_t


def _build_nc(dtype):
    mdt = _MY_DT[dtype]
    grp = 4 if dtype == "f16" else 2   # c-chunks per W DMA (~1MB tiles)
    nc = bacc.Bacc("TRN2", target_bir_lowering=False, debug=False)
    # x^T pre-tiled on host: xt[p, ch, b] = x[b, ch*128+p]
    xt_d = nc.dram_tensor("xt", [128, NCH * B], mdt, kind="ExternalInput")
    # W pre-tiled on host: w[p, ch, r] = W[ch*128+p, core_rows[r]]
    w_d = nc.dram_tensor("w", [128, NCH, RC], mdt, kind="ExternalInput")
    y_d = nc.dram_tensor("y", [128, RC], mybir.dt.float32, kind="ExternalOutput")

    with tile.TileContext(nc) as tc:
        with (
            tc.tile_pool(name="xsb", bufs=1) as xpool,
            tc.tile_pool(name="wsb", bufs=6) as wpool,
            tc.tile_pool(name="ysb", bufs=1) as ypool,
            tc.tile_pool(name="ps", bufs=NT, space="PSUM") as ppool,
        ):
            x_t = xpool.tile([128, NCH, B], mdt)
            # x^T loads are split and interleaved with the W stream below so
            # the first matmuls start as soon as slice 0 lands
            nxs = 8
            xs = NCH // nxs
            xt_issued = 0

            def _load_xt_upto(ch_needed):
                nonlocal xt_issued
                while xt_issued * xs <= ch_needed and xt_issued < nxs:
                    s = xt_issued
                    nc.sync.dma_start(
                        out=x_t[:, s * xs:(s + 1) * xs, :],
                        in_=xt_d[:, s * xs * B:(s + 1) * xs * B],
                    )
                    xt_issued += 1
            psums = [
                ppool.tile(
                    [128, 512], mybir.dt.float32, name=f"psum{t}", tag=f"psum{t}"
                )
                for t in range(NT)
            ]
            for g in range(NCH // grp):
                _load_xt_upto(min((g + 2) * grp, NCH - 1))
                w_t = wpool.tile([128, grp, RC], mdt)
                nc.sync.dma_start(
                    out=w_t[:], in_=w_d[:, g * grp:(g + 1) * grp, :]
                )
                for i in range(grp):
                    ch = g * grp + i
                    for t in range(NT):
                        nc.tensor.matmul(
                            psums[t][:],
                            x_t[:, ch, :],
                            w_t[:, i, t * 512:(t + 1) * 512],
                            start=(ch == 0),
                            stop=(ch == NCH - 1),
                        )
            y_t = ypool.tile([128, RC], mybir.dt.float32)
            for t in range(NT):
                nc.vector.tensor_copy(
                    out=y_t[:, t * 512:(t + 1) * 512], in_=psums[t][:]
                )
            nc.sync.dma_start(out=y_d[:], in_=y_t[:])
    nc.compile()
    return nc


_CACHE = {}
_TRACE = False  # set by bench harness to capture an NTFF profile


def _get_nc(dtype):
    if dtype not in _CACHE:
        _CACHE[dtype] = _build_nc(dtype)
    return _CACHE[dtype]


def kernel(x_batched, M_vals, M_row_idx, M_col_idx, _want_results=False, **_):
    x = np.asarray(x_batched, dtype=np.float32)
    vals = np.asarray(M_vals, dtype=np.float32)
    rows = np.asarray(M_row_idx, dtype=np.int64)
    cols = np.asarray(M_col_idx, dtype=np.int64)
    ndt = _NP_DT[DTYPE]

    w_t = _densify_tiled(vals, rows, cols).astype(ndt)   # [128, NCH, R]
    xt = np.ascontiguousarray(
        x.T.reshape(NCH, 128, B).transpose(1, 0, 2).reshape(128, NCH * B)
    ).astype(ndt)

    nc = _get_nc(DTYPE)
    in_maps = [
        {
            "xt": xt,
            "w": np.ascontiguousarray(w_t[:, :, m * RC:(m + 1) * RC]),
        }
        for m in range(NCORES)
    ]
    res = run_bass_kernel_spmd(
        nc, in_maps, core_ids=list(range(NCORES)), trace=_TRACE
    )

    y = np.empty((B, R), dtype=np.float32)
    for m in range(NCORES):
        y[:, m * RC:(m + 1) * RC] = res.results[m]["y"]
    if _want_results:
        return y, res
    return y


# revision 21
# speedup vs baseline: 1.1507x; 1.1507x over previous
"""Batched sparse-dense matmul (COO SpMM) on 8 Trainium2 NeuronCores.

Problem: y[b, r] = sum_k vals[k] * x[b, cols[k]] where rows[k] == r.
  x: [128, 16384] f32, vals/rows/cols: [524288], y: [128, 8192] f32.

Strategy: at 0.39% density with a full 128-wide batch, a dense matmul
y = x @ M^T beats any per-nonzero gather on this hardware (SWDGE
descriptor generation costs ~4-9ns per gathered element — a ~300us
serial floor on the Q7 cores — while the dense stream uses the HWDGE
DMA path with no per-element work at all).  So:
  - Host: densify M^T into W [C, R] (a format conversion of the matrix,
    analogous to CSR/ELL packing), shard W's output columns across the
    8 cores (1024 rows each), and pre-tile both x^T and W for the SBUF
    partition layout.  W and x are cast to fp16 (11-bit mantissa): the
    result error is ~3e-4 relative, and the stream halves vs f32.
  - Device (per core): keep x^T resident in SBUF as 128 [128c x 128b]
    chunks (the matmul's stationary operand); stream W from HBM in 1MB
    tiles (4 c-chunks each); PSUM accumulates over the 128 c-chunks
    into y[128b x 1024r] (fp32 accumulation); copy out via DVE.
  - Host: concatenate the per-core row slices.

Set DTYPE = "f32" for an exact (2e-5 absmax) variant at ~2x the time.
"""

import sys

sys.path.insert(0, "/opt/trn_rl_repo")

import numpy as np

import concourse.bacc as bacc
import concourse.mybir as mybir
import concourse.tile as tile
from concourse.bass_utils import run_bass_kernel_spmd

B = 128        # batch
R = 8192       # rows of sparse matrix / output features
C = 16384      # cols of sparse matrix / input features
NCORES = 8
RC = R // NCORES       # rows (output features) per core
NCH = C // 128         # contraction chunks of 128
NT = RC // 512         # 512-wide PSUM column tiles per core

DTYPE = "f16"          # "f16" (fast, ~3e-4 rel err) or "f32" (exact)
_NP_DT = {"f16": np.float16, "f32": np.float32}
_MY_DT = {"f16": mybir.dt.float16, "f32": mybir.dt.float32}


def _densify_tiled(vals, rows, cols):
    """w_t[p, ch, r] = sum of vals at (row=r, col=ch*128+p): dense M^T
    pre-tiled for the SBUF partition layout, [128, NCH, R] f32."""
    w_t = np.zeros((128, NCH, R), dtype=np.float32)
    np.add.at(w_t, (cols % 128, cols // 128, rows), vals)
    return w_t


def _build_nc(dtype):
    mdt = _MY_DT[dtype]
    grp = 4 if dtype == "f16" else 2   # c-chunks per W DMA (~1MB tiles)
    nc = bacc.Bacc("TRN2", target_bir_lowering=False, debug=False)
    # x^T pre-tiled on host: xt[p, ch, b] = x[b, ch*128+p]
    xt_d = nc.dram_tensor("xt", [128, NCH * B], mdt, kind="ExternalInput")
    # W pre-tiled on host: w[p, ch, r] = W[ch*128+p, core_rows[r]]
    w_d = nc.dram_tensor("w", [128, NCH, RC], mdt, kind="ExternalInput")
    y_d = nc.dram_tensor("y", [128, RC], mybir.dt.float32, kind="ExternalOutput")

    with tile.TileContext(nc) as tc:
        with (
            tc.tile_pool(name="xsb", bufs=1) as xpool,
            tc.tile_pool(name="wsb", bufs=6) as wpool,
            tc.tile_pool(name="ysb", bufs=1) as ypool,
            tc.tile_pool(name="ps", bufs=NT, space="PSUM") as ppool,
        ):
            x_t = xpool.tile([128, NCH, B], mdt)
            # x^T loads are split and interleaved with the W stream below so
            # the first matmuls start as soon as slice 0 lands
            nxs = 8
            xs = NCH // nxs
            xt_issued = 0

            def _load_xt_upto(ch_needed):
                nonlocal xt_issued
                while xt_issued * xs <= ch_needed and xt_issued < nxs:
                    s = xt_issued
                    nc.scalar.dma_start(
                        out=x_t[:, s * xs:(s + 1) * xs, :],
                        in_=xt_d[:, s * xs * B:(s + 1) * xs * B],
                    )
                    xt_issued += 1
            psums = [
                ppool.tile(
                    [128, 512], mybir.dt.float32, name=f"psum{t}", tag=f"psum{t}"
                )
                for t in range(NT)
            ]
            for g in range(NCH // grp):
                _load_xt_upto(min((g + 2) * grp, NCH - 1))
                w_t = wpool.tile([128, grp, RC], mdt)
                nc.sync.dma_start(
                    out=w_t[:], in_=w_d[:, g * grp:(g + 1) * grp, :]
                )
                for i in range(grp):
                    ch = g * grp + i
                    for t in range(NT):
                        nc.tensor.matmul(
                            psums[t][:],
                            x_t[:, ch, :],
                            w_t[:, i, t * 512:(t + 1) * 512],
                            start=(ch == 0),
                            stop=(ch == NCH - 1),
                        )
            y_t = ypool.tile([128, RC], mybir.dt.float32)
            for t in range(NT):
                nc.vector.tensor_copy(
                    out=y_t[:, t * 512:(t + 1) * 512], in_=psums[t][:]
                )
            nc.sync.dma_start(out=y_d[:], in_=y_t[:])
    nc.compile()
    return nc


_CACHE = {}
_TRACE = False  # set by bench harness to capture an NTFF profile


def _get_nc(dtype):
    if dtype not in _CACHE:
        _CACHE[dtype] = _build_nc(dtype)
    return _CACHE[dtype]


def kernel(x_batched, M_vals, M_row_idx, M_col_idx, _want_results=False, **_):
    x = np.asarray(x_batched, dtype=np.float32)
    vals = np.asarray(M_vals, dtype=np.float32)
    rows = np.asarray(M_row_idx, dtype=np.int64)
    cols = np.asarray(M_col_idx, dtype=np.int64)
    ndt = _NP_DT[DTYPE]

    w_t = _densify_tiled(vals, rows, cols).astype(ndt)   # [128, NCH, R]
    xt = np.ascontiguousarray(
        x.T.reshape(NCH, 128, B).transpose(1, 0, 2).reshape(128, NCH * B)
    ).astype(ndt)

    nc = _get_nc(DTYPE)
    in_maps = [
        {
            "xt": xt,
            "w": np.ascontiguousarray(w_t[:, :, m * RC:(m + 1) * RC]),
        }
        for m in range(NCORES)
    ]
    res = run_bass_kernel_spmd(
        nc, in_maps, core_ids=list(range(NCORES)), trace=_TRACE
    )

    y = np.empty((B, R), dtype=np.float32)
    for m in range(NCORES):
        y[:, m * RC:(m + 1) * RC] = res.results[m]["y"]
    if _want_results:
        return y, res
    return y
